# revision 44
# baseline (speedup 1.0000x reference)
"""MultiHeadAttention Trainium2 Bass kernel, v3.

Problem: B=2, S=2048, D=768, H=12 heads, head_dim=64.
    q = x@Wq+bq; k = x@Wk+bk; v = x@Wv+bv   (per-head split)
    out = softmax(q k^T / 8) v, heads merged, @ Wo + bo

Sharding (8 cores): core c handles batch b=c//4 and 3 heads (c%4)*3..+3
(Megatron attention: column-split of Wq/Wk/Wv, row-split of Wo). Each core
produces a partial [S, D] output; the host sums the 4 partials per batch and
adds (bv @ Wo + bo) once.

v3 over v2 (126.6us): v2 was paced by the ACT exp stream (96 x [128,1024]
exps at (1024+222)cyc @1.2GHz = 99.7us busy) plus a hard sc->exp psum
coupling (scores pool bufs=2 meant sc(i+2) waited exp(i)). v3:
 - offloads a tunable share of exp tiles to the DVE via a Schraudolph
   bit-trick exp (one tensor_scalar into an int16 tile whose bits ARE the
   bf16 exp; max rel err ~4.3%, pipeline rel err ~7e-3 vs the 2e-2 gate);
 - moves to 512-wide windows: 12 units x 16 key-tiles = 192 steps, scores
   psum [128,512] with bufs=4 so sc leads exp by 3 steps and exp leads ctx
   by 3 — the PE stream (~97us busy) becomes the pacer;
 - PSUM: 4x scores bank + 2x ctx bank + 2x filler bank = 8 banks; the
   out-projection borrows scores-ring slots (512+256 col chunks).

dtypes: moving-operand dtype sets matmul speed (f32r needs N>=256 for
1cyc/row; bf16 is 1cyc/row at any N). qT/kT stay f32r (full fp32 data);
x/v/ctx/Wv/Wo/identity are bf16 (small-N matmuls).

kernel(**inputs) takes FULL unsharded inputs and returns the FULL output.
"""

import numpy as np

import concourse.bass as bass
import concourse.mybir as mybir
import concourse.tile as tile
from concourse import bacc
from concourse.bass_utils import run_bass_kernel_spmd

F32 = mybir.dt.float32
F32R = mybir.dt.float32r
BF16 = mybir.dt.bfloat16
I16 = mybir.dt.int16

# Schraudolph exp on DVE: i16 = rint(s*SCH_C1 + SCH_C2); bitcast i16->bf16
# gives 2^n*(1+f) ~ exp(s) with max rel err ~4.3% (geometrically centered).
# DVE f32->i16 conversion is round-to-nearest (verified on hw).
SCH_C1 = 1.4426950408889634 * 128.0
SCH_C2 = 127.0 * 128.0 - 7.70

B, S, D = 2, 2048, 768
H, DH = 12, 64
NCORES = 8
HPC = 3                # heads per core
DH3 = HPC * DH         # 192 (per-core slice of the model dim)
KT = D // 128          # 6 contraction tiles for D
ST = S // 128          # 16 sequence tiles
GW = 512               # attention q-window width
NW = S // GW           # 4 windows
NSTEP = HPC * NW * ST  # 192 pipeline steps

_CACHED_NC = None


def _build_nc(debug: bool = False) -> bass.Bass:
    nc = bacc.Bacc()

    xT = nc.dram_tensor("xT", [D, S], BF16, kind="ExternalInput")
    wq = nc.dram_tensor("wq", [128, KT * 128], BF16, kind="ExternalInput")
    wk = nc.dram_tensor("wk", [128, KT * 128], BF16, kind="ExternalInput")
    wkq2 = nc.dram_tensor("wkq2", [128, KT * 128], BF16, kind="ExternalInput")
    wv = nc.dram_tensor("wv", [128, KT * DH3], BF16, kind="ExternalInput")
    wo = nc.dram_tensor("wo", [DH3, D], BF16, kind="ExternalInput")
    bias = nc.dram_tensor("bias", [128, 4], F32, kind="ExternalInput")
    ident = nc.dram_tensor("ident", [128, 128], BF16, kind="ExternalInput")
    out = nc.dram_tensor("out", [S, D], BF16, kind="ExternalOutput")

    with (
        tile.TileContext(nc) as tc,
        tc.tile_pool(name="big", bufs=1) as big,
        tc.tile_pool(name="work", bufs=2) as work,
        tc.tile_pool(name="expp", bufs=4) as expp,
        tc.tile_pool(name="outp", bufs=4) as outp,
        tc.tile_pool(name="psS", bufs=2, space="PSUM") as psS,
        tc.tile_pool(name="psB", bufs=2, space="PSUM") as psB,
        tc.tile_pool(name="psF", bufs=2, space="PSUM") as psF,
    ):
        # ---- persistent SBUF tensors ----
        x_sb = big.tile([128, KT, S], BF16)          # xT: [p, ktile, s]
        wq_sb = big.tile([128, KT, 128], BF16)
        wk_sb = big.tile([128, KT, 128], BF16)
        wkq2_sb = big.tile([128, KT, 128], BF16)     # [k_h2 | q_h2]
        wv_sb = big.tile([128, KT, DH3], BF16)
        woA_sb = big.tile([128, D], BF16)            # Wo rows 0..127
        woB_sb = big.tile([64, D], BF16)             # Wo rows 128..191
        bias_sb = big.tile([128, 4], F32)
        ident_sb = big.tile([128, 128], BF16)
        qTA = big.tile([128, S], F32R)               # q^T heads 0,1
        kTA = big.tile([128, S], F32R)
        qTB = big.tile([128, S], F32R)               # head 2 in rows 64:128
        kTB = big.tile([128, S], F32R)
        v_sb = big.tile([128, ST, HPC, DH + 1], BF16)  # v rows + ones col
        ctx_sb = big.tile([128, ST, DH3], BF16)      # [q-part, qt, h*64+d]
        ctxTA = big.tile([128, S], BF16)             # ctx^T heads 0,1
        ctxTB = big.tile([64, S], BF16)              # ctx^T head 2

        # ---- DMA loads ----
        # single sync queue: the serial (~360GB/s) DMA bus moves bytes in
        # exactly the order the pipeline consumes them. First window needs
        # wq + x(0:512) + wk + wv (v fillers start at step 0).
        nc.sync.dma_start(out=wq_sb, in_=wq.rearrange("p (kt m) -> p kt m", kt=KT))
        for c in range(2):
            cs = slice(c * 256, (c + 1) * 256)
            nc.sync.dma_start(
                out=x_sb[:, :, cs],
                in_=xT[:, cs].rearrange("(kt p) q -> p kt q", p=128),
            )
        nc.scalar.dma_start(out=bias_sb, in_=bias[:, :])
        nc.sync.dma_start(out=wk_sb, in_=wk.rearrange("p (kt m) -> p kt m", kt=KT))
        nc.sync.dma_start(out=wv_sb, in_=wv.rearrange("p (kt m) -> p kt m", kt=KT))
        nc.sync.dma_start(
            out=x_sb[:, :, 512:1024],
            in_=xT[:, 512:1024].rearrange("(kt p) q -> p kt q", p=128),
        )
        nc.sync.dma_start(out=wkq2_sb, in_=wkq2.rearrange("p (kt m) -> p kt m", kt=KT))
        for c in range(2, 4):
            cs = slice(c * 512, (c + 1) * 512)
            nc.sync.dma_start(
                out=x_sb[:, :, cs],
                in_=xT[:, cs].rearrange("(kt p) q -> p kt q", p=128),
            )
        nc.sync.dma_start(out=woA_sb, in_=wo[0:128, :])
        nc.sync.dma_start(out=woB_sb, in_=wo[128:DH3, :])
        nc.sync.dma_start(out=ident_sb, in_=ident[:, :])
        nc.vector.memset(v_sb[:, :, :, DH : DH + 1], 1.0)

        # preload the Exp activation table during the DMA wait (the implicit
        # LoadActFuncSet lands before this dummy, off the critical path)
        dum_in = big.tile([1, 1], F32)
        dum_out = big.tile([1, 1], F32)
        nc.vector.memset(dum_in, 0.0)
        nc.scalar.activation(dum_out, dum_in, mybir.ActivationFunctionType.Exp)

        # warm the PE p-state during the x DMA wait: the Tensor engine ramps
        # 0.65 -> 1.2 -> 2.4 GHz only after ~3us of continuous execution and
        # the ramp resets when PE idles; burn the DMA wait on throwaway
        # matmuls sized to end as the first x chunk lands (~1.6us).
        warm = big.tile([128, 512], BF16)
        nc.gpsimd.memset(warm, 0.0)   # Pool is idle; DVE memsets would delay
        for _ in range(5):
            pw = psS.tile([128, 1024], F32, tag="s", name="ps_warm")
            for hh in range(2):
                nc.tensor.matmul(pw[:, hh * 512 : (hh + 1) * 512],
                                 lhsT=warm[:, 0:128], rhs=warm, start=True,
                                 stop=True)

        # ---- projection helpers ----
        def proj_qk(pool, w_sb, cs, evict):
            n = cs.stop - cs.start
            ps = pool.tile([128, n], F32, tag=("s" if pool is psS else "f"),
                           name="ps_qk", padded_shape=None)
            for kt in range(KT):
                nc.tensor.matmul(
                    ps,
                    lhsT=w_sb[:, kt, :],
                    rhs=x_sb[:, kt, cs],
                    start=(kt == 0),
                    stop=(kt == KT - 1),
                )
            evict(ps, cs)

        def ev_k(ps, cs):
            nc.vector.tensor_scalar_add(kTA[:, cs], ps, bias_sb[:, 0:1])

        def ev_q(ps, cs):
            nc.vector.tensor_scalar_add(qTA[:, cs], ps, bias_sb[:, 2:3])

        def ev_kq2(ps, cs):
            # psum rows 0:64 = k_h2 (up-shift to 64:128), rows 64:128 = q_h2
            nc.vector.tensor_scalar_add(kTB[64:128, cs], ps[0:64, :], bias_sb[0:64, 1:2])
            nc.vector.tensor_scalar_add(qTB[64:128, cs], ps[64:128, :], bias_sb[64:128, 3:4])

        def proj_v(st):
            # all 3 heads' v for one seq tile: [128, 192] psum, one strided
            # eviction into v_sb (skipping the ones column)
            ss = slice(st * 128, (st + 1) * 128)
            ps_v = psF.tile([128, DH3], F32, tag="f", name="ps_v")
            for kt in range(KT):
                nc.tensor.matmul(
                    ps_v,
                    lhsT=x_sb[:, kt, ss],
                    rhs=wv_sb[:, kt, :],
                    start=(kt == 0),
                    stop=(kt == KT - 1),
                )
            nc.vector.tensor_copy(
                v_sb[:, st, :, 0:DH],
                ps_v.rearrange("p (h d) -> p h d", h=HPC),
            )

        # head h (q/k)^T slices: heads 0,1 in kTA/qTA rows 0:64 / 64:128,
        # head 2 in kTB/qTB rows 64:128.
        def kq_rows(h):
            if h == 0:
                return kTA, qTA, slice(0, 64)
            if h == 1:
                return kTA, qTA, slice(64, 128)
            return kTB, qTB, slice(64, 128)

        # ---- attention pipeline pieces ----
        # units of (head, 512-q-window); per unit j walks the 16 key tiles.
        # Heads in order (0,1,2) per window so h0h1's combined transpose of
        # the LAST window fires before the final unit (h2) completes.
        units = [(h, w) for w in range(NW) for h in range(HPC)]
        seq = [(h, w, j) for (h, w) in units for j in range(ST)]

        # scores/exp at PAIR granularity: two consecutive steps share one
        # [128,1024] psum tile and ONE exp instruction, halving the
        # per-instruction access overhead on the exp engines.
        sc_pairs = {}   # pair m -> scores psum tile (steps 2m, 2m+1)
        et_pairs = {}   # pair m -> exp tile
        ctx_ps = {}     # (h, w) -> ctx psum tile

        def sc_step(i):
            h, w, j = seq[i]
            m, half = i // 2, i % 2
            kk, qq, rows = kq_rows(h)
            if half == 0:
                sc_pairs[m] = psS.tile([128, 2 * GW], F32, tag="s",
                                       name="ps_sc")
            nc.tensor.matmul(
                sc_pairs[m][:, half * GW : (half + 1) * GW],
                lhsT=kk[rows, j * 128 : (j + 1) * 128],
                rhs=qq[rows, w * GW : (w + 1) * GW],
                start=True,
                stop=True,
            )

        def exp_pair(m, eng="act"):
            ps = sc_pairs.pop(m)
            et = expp.tile([128, 2 * GW], BF16, tag="e", name="expT")
            if eng == "dve":
                # approximate exp on DVE (bit-trick), freeing ACT: the int16
                # write IS the bf16 exp tile by bitcast.
                nc.vector.tensor_scalar(
                    out=et.bitcast(I16), in0=ps,
                    scalar1=SCH_C1, scalar2=SCH_C2,
                    op0=mybir.AluOpType.mult, op1=mybir.AluOpType.add,
                )
            else:
                nc.scalar.activation(et, ps, mybir.ActivationFunctionType.Exp)
            et_pairs[m] = et

        def ctx_step(i):
            h, w, j = seq[i]
            et = et_pairs[i // 2]
            e0 = (i % 2) * 4
            if i % 2 == 1:
                del et_pairs[i // 2]
            key = (h, w)
            if key not in ctx_ps:
                ctx_ps[key] = psB.tile([128, 4 * (DH + 1)], F32, tag="b",
                                       name="ps_ctx")
            pc = ctx_ps[key]
            for qq in range(4):
                # start marks the WHOLE 2KB psum bank pending-zero, so only
                # the first write of the bank's group may set it.
                nc.tensor.matmul(
                    pc[:, qq * (DH + 1) : (qq + 1) * (DH + 1)],
                    lhsT=et[:, (e0 + qq) * 128 : (e0 + qq + 1) * 128],
                    rhs=v_sb[:, j, h, :],
                    start=(j == 0 and qq == 0),
                    stop=(j == ST - 1 and qq == 3),
                    skip_group_check=True,
                )

        def norm_evict(h, w, last=False):
            # psum [128, 4*(65)]: per qq, cols 0:64 = ctx, col 64 = denom.
            pc = ctx_ps.pop((h, w))
            v3 = pc.rearrange("p (qq c) -> p qq c", c=DH + 1)
            rcp = work.tile([128, 4], F32, tag="rcp", name="rcp")
            nc.vector.reciprocal_approx_fast(
                out=rcp, in_=v3[:, :, DH : DH + 1].squeeze(-1))
            qt0 = w * 4
            if last:
                # final unit: ACT is idle after the last exp — normalize
                # there (Copy with per-partition scale), one qq per instr,
                # in parallel with DVE work
                for qq in range(4):
                    nc.scalar.activation(
                        ctx_sb[:, qt0 + qq, h * DH : (h + 1) * DH],
                        v3[:, qq, 0:DH],
                        mybir.ActivationFunctionType.Copy,
                        scale=rcp[:, qq : qq + 1],
                    )
                return
            nc.vector.tensor_mul(
                ctx_sb[:, qt0 : qt0 + 4, h * DH : (h + 1) * DH],
                v3[:, :, 0:DH],
                rcp.unsqueeze(-1).broadcast_to([128, 4, DH]),
            )

        # ---- transpose + out-projection ----
        def trans_ab(w, piece):
            # heads 0+1 combined: ctx_sb[:, qt, 0:128] is [q, h0 d | h1 d];
            # one [128,128] PE transpose per qt covers both heads. Two qt
            # per 256-col psF piece.
            pt = psF.tile([128, 256], BF16, tag="f", name="ps_t")
            for qq in (0, 1):
                qt = w * 4 + piece * 2 + qq
                nc.tensor.transpose(
                    pt[:, qq * 128 : (qq + 1) * 128],
                    ctx_sb[:, qt, 0:128], ident_sb,
                )
            cs = slice(w * GW + piece * 256, w * GW + (piece + 1) * 256)
            nc.vector.tensor_copy(ctxTA[:, cs], pt)

        def trans_c(w, piece):
            # head 2 -> ctxTB rows 0:64
            pt = psF.tile([64, 256], BF16, tag="f", name="ps_t2")
            for qq in (0, 1):
                qt = w * 4 + piece * 2 + qq
                nc.tensor.transpose(
                    pt[0:64, qq * 128 : (qq + 1) * 128],
                    ctx_sb[:, qt, 2 * DH : DH3], ident_sb,
                )
            cs = slice(w * GW + piece * 256, w * GW + (piece + 1) * 256)
            nc.vector.tensor_copy(ctxTB[0:64, cs], pt[0:64, :])

        o_tiles = {}

        NOC = 3   # out-projection chunks per qt (256 cols each, via psF)

        def outproj_chunk(qt, ci, eng="dve", bfirst=False, dma=None):
            # one 256-col psF chunk; two matmuls (K=192 over ctxTA 128 rows
            # + ctxTB 64 rows); evict into the qt's [128,768] staging tile;
            # the LAST chunk issues a single DMA for the qt (each DMA costs
            # ~625ns serialized HWDGE overhead, so one per qt).
            ss = slice(qt * 128, (qt + 1) * 128)
            osl = slice(ci * 256, (ci + 1) * 256)
            if qt not in o_tiles:
                o_tiles[qt] = outp.tile([128, D], BF16, tag="o", name="o_sb")
            o_sb = o_tiles[qt]
            po = psF.tile([128, 256], F32, tag="f", name="ps_o")
            pairs = [(ctxTA, woA_sb), (ctxTB, woB_sb)]
            if bfirst:
                pairs.reverse()
            for pi, (ct, wo_sb) in enumerate(pairs):
                nc.tensor.matmul(
                    po, lhsT=ct[:, qt * 128 : (qt + 1) * 128],
                    rhs=wo_sb[:, osl], start=(pi == 0), stop=(pi == 1),
                )
            if eng == "act":
                nc.scalar.activation(o_sb[:, osl], po,
                                     mybir.ActivationFunctionType.Copy)
            else:
                nc.vector.tensor_copy(o_sb[:, osl], po)
            if dma == "split":
                # final qt: DMA each chunk as its eviction lands so the
                # last serialized HWDGE+transfer is the small 256-col one
                nc.sync.dma_start(out=out[ss, osl], in_=o_sb[:, osl])
                if ci == NOC - 1:
                    del o_tiles[qt]
            elif ci == NOC - 1:
                del o_tiles[qt]
                if dma == "pool":
                    # SWDGE path — bypasses the single-slot HWDGE so the
                    # final DMAs drain in parallel
                    nc.gpsimd.dma_start(out=out[ss, :], in_=o_sb)
                else:
                    nc.sync.dma_start(out=out[ss, :], in_=o_sb)

        def outproj_qt(qt, eng="dve", bfirst=False, dma=None):
            if eng == "both":
                ee = ("dve", "act", "dve")
            elif eng == "rboth":
                ee = ("act", "dve", "act")
            else:
                ee = (eng, eng, eng)
            for ci in range(NOC):
                outproj_chunk(qt, ci, ee[ci], bfirst, dma)

        # ---- startup: projections needed before the exp stream starts ----
        # q w0 in two 256-col pieces pipelined behind the x DMA pieces;
        # k j0 next; the rest stream in as fillers.
        proj_qk(psF, wq_sb, slice(0, 256), ev_q)
        proj_qk(psF, wq_sb, slice(256, 512), ev_q)
        proj_qk(psF, wk_sb, slice(0, 128), ev_k)

        # ---- per-step filler schedule ----
        # sched[i] = closures emitted at pipeline step i. EMISSION-ORDER
        # LAW: a consumer emitted before its producer gets NO dependency
        # edge and reads garbage. Need-by rules:
        #   k chunk j        -> before step j-2 (sc(j) emitted at step j-3)
        #   v tile st        -> before step st (ctx(h0-unit, j=st))
        #   kq2 chunk c      -> k2 col j read at step 32+j; q2 w read at
        #                       step 48w+32 (chunks 4w..4w+3)
        #   q w chunks       -> before step 48w-3 (sc emission lead)
        #   trans w          -> after norm(1,w) at step 48w+31 (ab) and
        #                       norm(2,w) at step 48w+47 (c)
        #   outproj qt of w  -> after trans w
        def mk_qk(w_sb, cs, evict):
            return lambda: proj_qk(psF, w_sb, cs, evict)

        k2cs = [slice(128 * i, 128 * (i + 1)) for i in range(16)]
        sched = {}

        def put(slot, f):
            sched.setdefault(slot, []).append(f)

        for j in range(5, 16):   # k chunks for sc j=5..15 (j<5 pre-loop)
            put(j - 5, mk_qk(wk_sb, slice(j * 128, (j + 1) * 128), ev_k))
        for st in range(2, 16):  # v tiles (all 3 heads each)
            put(min(st - 2, 13), lambda st=st: proj_v(st))
        # kq2: q2 w0 (chunks 0..3) read at step 29; k2 col j at step 29+j —
        # spread the later chunks to fill the otherwise-empty steps 36..42
        K2SLOTS = [15, 16, 17, 18, 20, 22, 24, 26, 28, 30, 32, 34, 36, 38, 40, 42]
        for c in range(16):
            put(K2SLOTS[c], mk_qk(wkq2_sb, k2cs[c], ev_kq2))
        for w in range(1, NW):   # q for windows 1..3: 4 chunks each
            for c in range(4):
                cs = slice(w * GW + c * 128, w * GW + (c + 1) * 128)
                put(48 * w - 16 + 3 * c, mk_qk(wq_sb, cs, ev_q))
        # transposes + out-projection per window (windows 0..2 here; the
        # last window's h2 part and out-proj form the tail). Out-proj
        # chunks are spread so the filler stream reaches into the
        # otherwise-empty final unit.
        for w in range(NW):
            base = 48 * w
            for piece in range(2):
                # fill the otherwise-empty window-end steps 44..47 (norm of
                # (1,w) lands at step 48w+31, so 44/46 are safe); the last
                # window's blocks go right after its h1 norm instead.
                put(base + (44 + 2 * piece if w < NW - 1 else 32 + 2 * piece),
                    lambda w=w, p=piece: trans_ab(w, p))
                if w < NW - 1:
                    put(base + 48 + piece, lambda w=w, p=piece: trans_c(w, p))
            if w < NW - 1:
                step = 4 if w == NW - 2 else 3
                for kc in range(12):
                    qt, ci = w * 4 + kc // 3, kc % 3
                    put(base + 50 + step * kc,
                        lambda qt=qt, ci=ci, kc=kc:
                            outproj_chunk(qt, ci,
                                          eng=("act" if (kc // 3) % 2 else "dve")))

        # exp-engine assignment per PAIR: DVE takes a share sized to each
        # region's DVE eviction load.
        def exp_eng(m):
            if m < 8:
                return "act"        # unit 0: DVE busy with v/k evicts
            if m < 24:
                return "dve" if m % 2 == 1 else "act"
            if m >= 94:
                return "act"   # the final pairs gate the tail
            return "dve" if (m % 8) in (1, 3, 6) else "act"

        # ---- main pipelined emission ----
        # sc leads ctx by 4 steps; each pair's exp is emitted right after
        # its second sc so it sits early in its engine queue.
        LEAD = 4
        for i in range(LEAD):
            sc_step(i)
            if i % 2 == 1:
                exp_pair(i // 2, eng=exp_eng(i // 2))
            if i == 0:
                proj_qk(psF, wk_sb, slice(128, 256), ev_k)
            elif i == 1:
                proj_qk(psF, wk_sb, slice(256, 384), ev_k)
                proj_v(0)
            elif i == 2:
                proj_v(1)
                proj_qk(psF, wk_sb, slice(384, 512), ev_k)
            else:
                proj_qk(psF, wk_sb, slice(512, 640), ev_k)
        for i, (h, w, j) in enumerate(seq):
            ctx_step(i)
            if i + LEAD < NSTEP:
                sc_step(i + LEAD)
                if (i + LEAD) % 2 == 1:
                    exp_pair((i + LEAD) // 2, eng=exp_eng((i + LEAD) // 2))
            if j == ST - 1 and i != NSTEP - 1:
                norm_evict(h, w)
            for f in sched.pop(i, []):
                f()
        assert not sched, f"unemitted filler slots: {sorted(sched)}"

        # ---- tail: last unit's norm pipelined with the h2 transpose and
        # the out-projection of qt 12..15. The ACT norm copies are emitted
        # qq-pairwise so each transpose piece starts as soon as its two
        # ctx_sb subtiles exist; qt12/13 only need transpose piece 0.
        pc = ctx_ps.pop((2, NW - 1))
        v3 = pc.rearrange("p (qq c) -> p qq c", c=DH + 1)
        rcp = work.tile([128, 4], F32, tag="rcp", name="rcp")
        nc.vector.reciprocal_approx_fast(
            out=rcp, in_=v3[:, :, DH : DH + 1].squeeze(-1))
        qt0 = (NW - 1) * 4
        # halves on both engines concurrently: DVE mul (qq0,1) + ACT scaled
        # copies (qq2,3) — the first transpose piece only needs qq0,1
        nc.vector.tensor_mul(
            ctx_sb[:, qt0 : qt0 + 2, 2 * DH : DH3],
            v3[:, 0:2, 0:DH],
            rcp[:, 0:2].unsqueeze(-1).broadcast_to([128, 2, DH]),
        )
        for qq in (2, 3):
            nc.scalar.activation(
                ctx_sb[:, qt0 + qq, 2 * DH : DH3],
                v3[:, qq, 0:DH],
                mybir.ActivationFunctionType.Copy,
                scale=rcp[:, qq : qq + 1],
            )
        trans_c(NW - 1, 0)
        outproj_qt(12, eng="dve")
        trans_c(NW - 1, 1)
        outproj_qt(13, eng="act")
        outproj_qt(14, eng="both", bfirst=True, dma="pool")
        outproj_qt(15, eng="rboth", bfirst=True)

    nc.compile()
    return nc


def _w_rearrange(w):
    """[768, M] -> [128, 6*M] bf16: row p holds w[kt*128+p, :] for kt=0..5,
    so the device DMA is one contiguous segment per partition."""
    import ml_dtypes

    d, m = w.shape
    kt = d // 128
    return np.ascontiguousarray(
        w.reshape(kt, 128, m).transpose(1, 0, 2).reshape(128, kt * m)
    ).astype(ml_dtypes.bfloat16)


def _bias_block(bq, bk, col):
    # [128, 4]: col0 = bk heads01, col1 = bk head2 (rows 0:64),
    # col2 = bq heads01 (pre-scaled), col3 = bq head2 at rows 64:128
    blk = np.zeros((128, 4), np.float32)
    blk[:, 0] = bk[col : col + 128]
    blk[0:64, 1] = bk[col + 128 : col + 192]
    blk[:, 2] = bq[col : col + 128] * np.float32(0.125)
    blk[64:128, 3] = bq[col + 128 : col + 192] * np.float32(0.125)
    return blk


def _prep_in_maps(inputs):
    import ml_dtypes

    bf16 = ml_dtypes.bfloat16
    x = np.asarray(inputs["x"], dtype=np.float32)
    Wq = np.asarray(inputs["Wq"], dtype=np.float32)
    Wk = np.asarray(inputs["Wk"], dtype=np.float32)
    Wv = np.asarray(inputs["Wv"], dtype=np.float32)
    Wo = np.asarray(inputs["Wo"], dtype=np.float32)
    bq = np.asarray(inputs["bq"], dtype=np.float32)
    bk = np.asarray(inputs["bk"], dtype=np.float32)
    ident = np.eye(128, dtype=np.float32).astype(bf16)

    in_maps = []
    for c in range(NCORES):
        b = c // 4
        col = (c % 4) * DH3
        sl = slice(col, col + DH3)
        in_maps.append(
            {
                "xT": np.ascontiguousarray(x[b].T).astype(bf16),
                "wq": _w_rearrange(Wq[:, col : col + 128] * np.float32(0.125)),
                "wk": _w_rearrange(Wk[:, col : col + 128]),
                "wkq2": _w_rearrange(np.concatenate(
                    [
                        Wk[:, col + 128 : col + 192],
                        Wq[:, col + 128 : col + 192] * np.float32(0.125),
                    ],
                    axis=1,
                )),
                "wv": _w_rearrange(Wv[:, sl]),
                "wo": np.ascontiguousarray(Wo[sl, :]).astype(bf16),
                "bias": _bias_block(bq, bk, col),
                "ident": ident,
            }
        )
    return in_maps


def _combine(results, inputs):
    Wo = np.asarray(inputs["Wo"], dtype=np.float32)
    bv = np.asarray(inputs["bv"], dtype=np.float32)
    bo = np.asarray(inputs["bo"], dtype=np.float32)
    base = bv @ Wo + bo  # [D]
    out = np.empty((B, S, D), dtype=np.float32)
    for b in range(B):
        acc = results[4 * b]["out"].astype(np.float32)
        for c in range(4 * b + 1, 4 * b + 4):
            acc = acc + results[c]["out"].astype(np.float32)
        out[b] = acc + base
    return out


def run(inputs, trace: bool = False):
    """Run the 8-core kernel; returns (output, BassKernelResults)."""
    global _CACHED_NC
    if _CACHED_NC is None:
        _CACHED_NC = _build_nc()
    in_maps = _prep_in_maps(inputs)
    try:
        res = run_bass_kernel_spmd(
            _CACHED_NC, in_maps, core_ids=list(range(NCORES)), trace=trace
        )
    except ModuleNotFoundError:
        import os

        os.environ["BASS_NEVER_TRACE"] = "1"
        res = run_bass_kernel_spmd(
            _CACHED_NC, in_maps, core_ids=list(range(NCORES)), trace=False
        )
    return _combine(res.results, inputs), res


def kernel(**inputs) -> np.ndarray:
    out, _ = run(inputs)
    return out


# revision 45
# speedup vs baseline: 1.1679x; 1.1679x over previous
"""MultiHeadAttention Trainium2 Bass kernel, v3.

Problem: B=2, S=2048, D=768, H=12 heads, head_dim=64.
    q = x@Wq+bq; k = x@Wk+bk; v = x@Wv+bv   (per-head split)
    out = softmax(q k^T / 8) v, heads merged, @ Wo + bo

Sharding (8 cores): core c handles batch b=c//4 and 3 heads (c%4)*3..+3
(Megatron attention: column-split of Wq/Wk/Wv, row-split of Wo). Each core
produces a partial [S, D] output; the host sums the 4 partials per batch and
adds (bv @ Wo + bo) once.

v3 over v2 (126.6us): v2 was paced by the ACT exp stream (96 x [128,1024]
exps at (1024+222)cyc @1.2GHz = 99.7us busy) plus a hard sc->exp psum
coupling (scores pool bufs=2 meant sc(i+2) waited exp(i)). v3:
 - offloads a tunable share of exp tiles to the DVE via a Schraudolph
   bit-trick exp (one tensor_scalar into an int16 tile whose bits ARE the
   bf16 exp; max rel err ~4.3%, pipeline rel err ~7e-3 vs the 2e-2 gate);
 - moves to 512-wide windows: 12 units x 16 key-tiles = 192 steps, scores
   psum [128,512] with bufs=4 so sc leads exp by 3 steps and exp leads ctx
   by 3 — the PE stream (~97us busy) becomes the pacer;
 - PSUM: 4x scores bank + 2x ctx bank + 2x filler bank = 8 banks; the
   out-projection borrows scores-ring slots (512+256 col chunks).

dtypes: moving-operand dtype sets matmul speed (f32r needs N>=256 for
1cyc/row; bf16 is 1cyc/row at any N). qT/kT stay f32r (full fp32 data);
x/v/ctx/Wv/Wo/identity are bf16 (small-N matmuls).

kernel(**inputs) takes FULL unsharded inputs and returns the FULL output.
"""

import numpy as np

import concourse.bass as bass
import concourse.mybir as mybir
import concourse.tile as tile
from concourse import bacc
from concourse.bass_utils import run_bass_kernel_spmd

F32 = mybir.dt.float32
F32R = mybir.dt.float32r
BF16 = mybir.dt.bfloat16
I16 = mybir.dt.int16

# Schraudolph exp on DVE: i16 = rint(s*SCH_C1 + SCH_C2); bitcast i16->bf16
# gives 2^n*(1+f) ~ exp(s) with max rel err ~4.3% (geometrically centered).
# DVE f32->i16 conversion is round-to-nearest (verified on hw).
SCH_C1 = 1.4426950408889634 * 128.0
SCH_C2 = 127.0 * 128.0 - 7.70

B, S, D = 2, 2048, 768
H, DH = 12, 64
NCORES = 8
HPC = 3                # heads per core
DH3 = HPC * DH         # 192 (per-core slice of the model dim)
KT = D // 128          # 6 contraction tiles for D
ST = S // 128          # 16 sequence tiles
GW = 512               # attention q-window width
NW = S // GW           # 4 windows
NSTEP = HPC * NW * ST  # 192 pipeline steps

_CACHED_NC = None


def _build_nc(debug: bool = False) -> bass.Bass:
    nc = bacc.Bacc()

    xT = nc.dram_tensor("xT", [D, S], BF16, kind="ExternalInput")
    wq = nc.dram_tensor("wq", [128, KT * 128], BF16, kind="ExternalInput")
    wk = nc.dram_tensor("wk", [128, KT * 128], BF16, kind="ExternalInput")
    wkq2 = nc.dram_tensor("wkq2", [128, KT * 128], BF16, kind="ExternalInput")
    wv = nc.dram_tensor("wv", [128, KT * DH3], BF16, kind="ExternalInput")
    wo = nc.dram_tensor("wo", [DH3, D], BF16, kind="ExternalInput")
    bias = nc.dram_tensor("bias", [128, 4], F32, kind="ExternalInput")
    ident = nc.dram_tensor("ident", [128, 128], BF16, kind="ExternalInput")
    out = nc.dram_tensor("out", [S, D], BF16, kind="ExternalOutput")

    with (
        tile.TileContext(nc) as tc,
        tc.tile_pool(name="big", bufs=1) as big,
        tc.tile_pool(name="work", bufs=2) as work,
        tc.tile_pool(name="expp", bufs=4) as expp,
        tc.tile_pool(name="outp", bufs=4) as outp,
        tc.tile_pool(name="psS", bufs=4, space="PSUM") as psS,
        tc.tile_pool(name="psB", bufs=2, space="PSUM") as psB,
        tc.tile_pool(name="psF", bufs=2, space="PSUM") as psF,
    ):
        # ---- persistent SBUF tensors ----
        x_sb = big.tile([128, KT, S], BF16)          # xT: [p, ktile, s]
        wq_sb = big.tile([128, KT, 128], BF16)
        wk_sb = big.tile([128, KT, 128], BF16)
        wkq2_sb = big.tile([128, KT, 128], BF16)     # [k_h2 | q_h2]
        wv_sb = big.tile([128, KT, DH3], BF16)
        woA_sb = big.tile([128, D], BF16)            # Wo rows 0..127
        woB_sb = big.tile([64, D], BF16)             # Wo rows 128..191
        bias_sb = big.tile([128, 4], F32)
        ident_sb = big.tile([128, 128], BF16)
        qTA = big.tile([128, S], F32R)               # q^T heads 0,1
        kTA = big.tile([128, S], F32R)
        qTB = big.tile([128, S], F32R)               # head 2 in rows 64:128
        kTB = big.tile([128, S], F32R)
        v_sb = big.tile([128, ST, HPC, DH + 1], BF16)  # v rows + ones col
        ctx_sb = big.tile([128, ST, DH3], BF16)      # [q-part, qt, h*64+d]
        ctxTA = big.tile([128, S], BF16)             # ctx^T heads 0,1
        ctxTB = big.tile([64, S], BF16)              # ctx^T head 2

        # ---- DMA loads ----
        # single sync queue: the serial (~360GB/s) DMA bus moves bytes in
        # exactly the order the pipeline consumes them. First window needs
        # wq + x(0:512) + wk + wv (v fillers start at step 0).
        nc.sync.dma_start(out=wq_sb, in_=wq.rearrange("p (kt m) -> p kt m", kt=KT))
        for c in range(2):
            cs = slice(c * 256, (c + 1) * 256)
            nc.sync.dma_start(
                out=x_sb[:, :, cs],
                in_=xT[:, cs].rearrange("(kt p) q -> p kt q", p=128),
            )
        nc.scalar.dma_start(out=bias_sb, in_=bias[:, :])
        nc.sync.dma_start(out=wk_sb, in_=wk.rearrange("p (kt m) -> p kt m", kt=KT))
        nc.sync.dma_start(out=wv_sb, in_=wv.rearrange("p (kt m) -> p kt m", kt=KT))
        nc.sync.dma_start(
            out=x_sb[:, :, 512:1024],
            in_=xT[:, 512:1024].rearrange("(kt p) q -> p kt q", p=128),
        )
        nc.sync.dma_start(out=wkq2_sb, in_=wkq2.rearrange("p (kt m) -> p kt m", kt=KT))
        for c in range(2, 4):
            cs = slice(c * 512, (c + 1) * 512)
            nc.sync.dma_start(
                out=x_sb[:, :, cs],
                in_=xT[:, cs].rearrange("(kt p) q -> p kt q", p=128),
            )
        nc.sync.dma_start(out=woA_sb, in_=wo[0:128, :])
        nc.sync.dma_start(out=woB_sb, in_=wo[128:DH3, :])
        nc.sync.dma_start(out=ident_sb, in_=ident[:, :])
        nc.vector.memset(v_sb[:, :, :, DH : DH + 1], 1.0)

        # preload the Exp activation table during the DMA wait (the implicit
        # LoadActFuncSet lands before this dummy, off the critical path)
        dum_in = big.tile([1, 1], F32)
        dum_out = big.tile([1, 1], F32)
        nc.vector.memset(dum_in, 0.0)
        nc.scalar.activation(dum_out, dum_in, mybir.ActivationFunctionType.Exp)

        # warm the PE p-state during the x DMA wait: the Tensor engine ramps
        # 0.65 -> 1.2 -> 2.4 GHz only after ~3us of continuous execution and
        # the ramp resets when PE idles; burn the DMA wait on throwaway
        # matmuls sized to end as the first x chunk lands (~1.6us).
        warm = big.tile([128, 512], BF16)
        nc.gpsimd.memset(warm, 0.0)   # Pool is idle; DVE memsets would delay
        for _ in range(9):
            pw = psS.tile([128, 512], F32, tag="s", name="ps_warm")
            nc.tensor.matmul(pw, lhsT=warm[:, 0:128], rhs=warm, start=True,
                             stop=True)

        # ---- projection helpers ----
        def proj_qk(pool, w_sb, cs, evict):
            n = cs.stop - cs.start
            ps = pool.tile([128, n], F32, tag=("s" if pool is psS else "f"),
                           name="ps_qk", padded_shape=None)
            for kt in range(KT):
                nc.tensor.matmul(
                    ps,
                    lhsT=w_sb[:, kt, :],
                    rhs=x_sb[:, kt, cs],
                    start=(kt == 0),
                    stop=(kt == KT - 1),
                )
            evict(ps, cs)

        def ev_k(ps, cs):
            nc.vector.tensor_scalar_add(kTA[:, cs], ps, bias_sb[:, 0:1])

        def ev_q(ps, cs):
            nc.vector.tensor_scalar_add(qTA[:, cs], ps, bias_sb[:, 2:3])

        def ev_kq2(ps, cs):
            # psum rows 0:64 = k_h2 (up-shift to 64:128), rows 64:128 = q_h2
            nc.vector.tensor_scalar_add(kTB[64:128, cs], ps[0:64, :], bias_sb[0:64, 1:2])
            nc.vector.tensor_scalar_add(qTB[64:128, cs], ps[64:128, :], bias_sb[64:128, 3:4])

        def proj_v(st):
            # all 3 heads' v for one seq tile: [128, 192] psum, one strided
            # eviction into v_sb (skipping the ones column)
            ss = slice(st * 128, (st + 1) * 128)
            ps_v = psF.tile([128, DH3], F32, tag="f", name="ps_v")
            for kt in range(KT):
                nc.tensor.matmul(
                    ps_v,
                    lhsT=x_sb[:, kt, ss],
                    rhs=wv_sb[:, kt, :],
                    start=(kt == 0),
                    stop=(kt == KT - 1),
                )
            nc.vector.tensor_copy(
                v_sb[:, st, :, 0:DH],
                ps_v.rearrange("p (h d) -> p h d", h=HPC),
            )

        # head h (q/k)^T slices: heads 0,1 in kTA/qTA rows 0:64 / 64:128,
        # head 2 in kTB/qTB rows 64:128.
        def kq_rows(h):
            if h == 0:
                return kTA, qTA, slice(0, 64)
            if h == 1:
                return kTA, qTA, slice(64, 128)
            return kTB, qTB, slice(64, 128)

        # ---- attention pipeline pieces ----
        # units of (head, 512-q-window); per unit j walks the 16 key tiles.
        # Heads in order (0,1,2) per window so h0h1's combined transpose of
        # the LAST window fires before the final unit (h2) completes.
        units = [(h, w) for w in range(NW) for h in range(HPC)]
        seq = [(h, w, j) for (h, w) in units for j in range(ST)]

        sc_tiles = {}   # step -> scores psum tile
        et_tiles = {}   # step -> exp tile
        ctx_ps = {}     # (h, w) -> ctx psum tile

        def sc_step(i):
            h, w, j = seq[i]
            kk, qq, rows = kq_rows(h)
            ps = psS.tile([128, GW], F32, tag="s", name="ps_sc")
            nc.tensor.matmul(
                ps,
                lhsT=kk[rows, j * 128 : (j + 1) * 128],
                rhs=qq[rows, w * GW : (w + 1) * GW],
                start=True,
                stop=True,
            )
            sc_tiles[i] = ps

        def exp_step(i, eng="act"):
            ps = sc_tiles.pop(i)
            et = expp.tile([128, GW], BF16, tag="e", name="expT")
            if eng == "dve":
                # approximate exp on DVE (bit-trick), freeing ACT: the int16
                # write IS the bf16 exp tile by bitcast.
                nc.vector.tensor_scalar(
                    out=et.bitcast(I16), in0=ps,
                    scalar1=SCH_C1, scalar2=SCH_C2,
                    op0=mybir.AluOpType.mult, op1=mybir.AluOpType.add,
                )
            else:
                nc.scalar.activation(et, ps, mybir.ActivationFunctionType.Exp)
            et_tiles[i] = et

        def ctx_step(i):
            h, w, j = seq[i]
            et = et_tiles.pop(i)
            e0 = 0
            key = (h, w)
            if key not in ctx_ps:
                ctx_ps[key] = psB.tile([128, 4 * (DH + 1)], F32, tag="b",
                                       name="ps_ctx")
            pc = ctx_ps[key]
            for qq in range(4):
                # start marks the WHOLE 2KB psum bank pending-zero, so only
                # the first write of the bank's group may set it.
                nc.tensor.matmul(
                    pc[:, qq * (DH + 1) : (qq + 1) * (DH + 1)],
                    lhsT=et[:, (e0 + qq) * 128 : (e0 + qq + 1) * 128],
                    rhs=v_sb[:, j, h, :],
                    start=(j == 0 and qq == 0),
                    stop=(j == ST - 1 and qq == 3),
                    skip_group_check=True,
                )

        def norm_evict(h, w, last=False):
            # psum [128, 4*(65)]: per qq, cols 0:64 = ctx, col 64 = denom.
            pc = ctx_ps.pop((h, w))
            v3 = pc.rearrange("p (qq c) -> p qq c", c=DH + 1)
            rcp = work.tile([128, 4], F32, tag="rcp", name="rcp")
            nc.vector.reciprocal_approx_fast(
                out=rcp, in_=v3[:, :, DH : DH + 1].squeeze(-1))
            qt0 = w * 4
            if last:
                # final unit: ACT is idle after the last exp — normalize
                # there (Copy with per-partition scale), one qq per instr,
                # in parallel with DVE work
                for qq in range(4):
                    nc.scalar.activation(
                        ctx_sb[:, qt0 + qq, h * DH : (h + 1) * DH],
                        v3[:, qq, 0:DH],
                        mybir.ActivationFunctionType.Copy,
                        scale=rcp[:, qq : qq + 1],
                    )
                return
            nc.vector.tensor_mul(
                ctx_sb[:, qt0 : qt0 + 4, h * DH : (h + 1) * DH],
                v3[:, :, 0:DH],
                rcp.unsqueeze(-1).broadcast_to([128, 4, DH]),
            )

        # ---- transpose + out-projection ----
        def trans_ab(w, piece):
            # heads 0+1 combined: ctx_sb[:, qt, 0:128] is [q, h0 d | h1 d];
            # one [128,128] PE transpose per qt covers both heads. Two qt
            # per 256-col psF piece.
            pt = psF.tile([128, 256], BF16, tag="f", name="ps_t")
            for qq in (0, 1):
                qt = w * 4 + piece * 2 + qq
                nc.tensor.transpose(
                    pt[:, qq * 128 : (qq + 1) * 128],
                    ctx_sb[:, qt, 0:128], ident_sb,
                )
            cs = slice(w * GW + piece * 256, w * GW + (piece + 1) * 256)
            nc.vector.tensor_copy(ctxTA[:, cs], pt)

        def trans_c(w, piece):
            # head 2 -> ctxTB rows 0:64
            pt = psF.tile([64, 256], BF16, tag="f", name="ps_t2")
            for qq in (0, 1):
                qt = w * 4 + piece * 2 + qq
                nc.tensor.transpose(
                    pt[0:64, qq * 128 : (qq + 1) * 128],
                    ctx_sb[:, qt, 2 * DH : DH3], ident_sb,
                )
            cs = slice(w * GW + piece * 256, w * GW + (piece + 1) * 256)
            nc.vector.tensor_copy(ctxTB[0:64, cs], pt[0:64, :])

        o_tiles = {}

        def outproj_chunk(qt, ci, eng="dve", bfirst=False, dma=None):
            # one psS-ring chunk (512 or 256 cols); two matmuls (K=192 over
            # ctxTA 128 rows + ctxTB 64 rows); evict into the qt's [128,768]
            # staging tile; the SECOND chunk issues a single DMA for the qt
            # (each DMA costs ~625ns serialized HWDGE overhead, so one per
            # qt). Chunk-granular so the psS ring pressure per step stays
            # at one slot.
            ss = slice(qt * 128, (qt + 1) * 128)
            osl = (slice(0, 512), slice(512, D))[ci]
            if qt not in o_tiles:
                o_tiles[qt] = outp.tile([128, D], BF16, tag="o", name="o_sb")
            o_sb = o_tiles[qt]
            n = osl.stop - osl.start
            po = psS.tile([128, n], F32, tag="s", name="ps_o")
            pairs = [(ctxTA, woA_sb), (ctxTB, woB_sb)]
            if bfirst:
                pairs.reverse()
            for pi, (ct, wo_sb) in enumerate(pairs):
                nc.tensor.matmul(
                    po, lhsT=ct[:, qt * 128 : (qt + 1) * 128],
                    rhs=wo_sb[:, osl], start=(pi == 0), stop=(pi == 1),
                )
            if eng == "act":
                nc.scalar.activation(o_sb[:, osl], po,
                                     mybir.ActivationFunctionType.Copy)
            else:
                nc.vector.tensor_copy(o_sb[:, osl], po)
            if dma == "split":
                # final qt: DMA each chunk as its eviction lands so the
                # last serialized HWDGE+transfer is the small 256-col one
                nc.sync.dma_start(out=out[ss, osl], in_=o_sb[:, osl])
                if ci == 1:
                    del o_tiles[qt]
            elif ci == 1:
                del o_tiles[qt]
                if dma == "pool":
                    # SWDGE path — bypasses the single-slot HWDGE so the
                    # final DMAs drain in parallel
                    nc.gpsimd.dma_start(out=out[ss, :], in_=o_sb)
                else:
                    nc.sync.dma_start(out=out[ss, :], in_=o_sb)

        def outproj_qt(qt, eng="dve", bfirst=False, dma=None):
            if eng == "both":
                e0, e1 = "dve", "act"
            elif eng == "rboth":
                e0, e1 = "act", "dve"
            else:
                e0, e1 = eng, eng
            outproj_chunk(qt, 0, e0, bfirst, dma)
            outproj_chunk(qt, 1, e1, bfirst, dma)

        # ---- startup: projections needed before the exp stream starts ----
        # q w0 in two 256-col pieces pipelined behind the x DMA pieces;
        # k j0 next; the rest stream in as fillers.
        proj_qk(psS, wq_sb, slice(0, 256), ev_q)
        proj_qk(psS, wq_sb, slice(256, 512), ev_q)
        proj_qk(psS, wk_sb, slice(0, 128), ev_k)

        # ---- per-step filler schedule ----
        # sched[i] = closures emitted at pipeline step i. EMISSION-ORDER
        # LAW: a consumer emitted before its producer gets NO dependency
        # edge and reads garbage. Need-by rules:
        #   k chunk j        -> before step j-2 (sc(j) emitted at step j-3)
        #   v tile st        -> before step st (ctx(h0-unit, j=st))
        #   kq2 chunk c      -> k2 col j read at step 32+j; q2 w read at
        #                       step 48w+32 (chunks 4w..4w+3)
        #   q w chunks       -> before step 48w-3 (sc emission lead)
        #   trans w          -> after norm(1,w) at step 48w+31 (ab) and
        #                       norm(2,w) at step 48w+47 (c)
        #   outproj qt of w  -> after trans w
        def mk_qk(w_sb, cs, evict):
            return lambda: proj_qk(psF, w_sb, cs, evict)

        k2cs = [slice(128 * i, 128 * (i + 1)) for i in range(16)]
        sched = {}

        def put(slot, f):
            sched.setdefault(slot, []).append(f)

        for j in range(4, 16):   # k chunks for sc j=4..15 (j<4 pre-loop)
            put(j - 4, mk_qk(wk_sb, slice(j * 128, (j + 1) * 128), ev_k))
        for st in range(2, 16):  # v tiles (all 3 heads each)
            put(min(st - 2, 13), lambda st=st: proj_v(st))
        # kq2: q2 w0 (chunks 0..3) read at step 29; k2 col j at step 29+j —
        # spread the later chunks to fill the otherwise-empty steps 36..42
        K2SLOTS = [15, 16, 17, 18, 20, 22, 24, 26, 28, 30, 32, 34, 36, 38, 40, 42]
        for c in range(16):
            put(K2SLOTS[c], mk_qk(wkq2_sb, k2cs[c], ev_kq2))
        for w in range(1, NW):   # q for windows 1..3: 4 chunks each
            for c in range(4):
                cs = slice(w * GW + c * 128, w * GW + (c + 1) * 128)
                put(48 * w - 16 + 3 * c, mk_qk(wq_sb, cs, ev_q))
        # transposes + out-projection per window (windows 0..2 here; the
        # last window's h2 part and out-proj form the tail). Out-proj
        # chunks are spread so the filler stream reaches into the
        # otherwise-empty final unit.
        for w in range(NW):
            base = 48 * w
            for piece in range(2):
                # fill the otherwise-empty window-end steps 44..47 (norm of
                # (1,w) lands at step 48w+31, so 44/46 are safe); the last
                # window's blocks go right after its h1 norm instead.
                put(base + (44 + 2 * piece if w < NW - 1 else 32 + 2 * piece),
                    lambda w=w, p=piece: trans_ab(w, p))
                if w < NW - 1:
                    put(base + 48 + piece, lambda w=w, p=piece: trans_c(w, p))
            if w < NW - 1:
                step = 6 if w == NW - 2 else 4
                for kc in range(8):
                    qt, ci = w * 4 + kc // 2, kc % 2
                    put(base + 50 + step * kc,
                        lambda qt=qt, ci=ci, kc=kc:
                            outproj_chunk(qt, ci,
                                          eng=("act" if (kc // 2) % 2 else "dve")))

        # exp-engine assignment: DVE takes a share sized to each
        # region's DVE eviction load.
        def exp_eng(i):
            if i < 16:
                return "act"        # unit 0: DVE busy with v/k evicts
            if i < 48:
                return "dve" if i % 4 == 1 else "act"
            if i >= 189:
                return "dve" if i == 190 else "act"  # final steps gate the tail
            return "dve" if (i % 16) in (1, 3, 5, 8, 10, 12, 14) else "act"

        # ---- main pipelined emission ----
        # sc/exp lead ctx by 3 steps (psS ring depth 4); exp is emitted
        # right after its sc so it sits early in its engine queue.
        LEAD = 3
        for i in range(LEAD):
            sc_step(i)
            exp_step(i, eng=exp_eng(i))
            if i == 0:
                proj_qk(psF, wk_sb, slice(128, 256), ev_k)
            elif i == 1:
                proj_qk(psF, wk_sb, slice(256, 384), ev_k)
                proj_v(0)
            else:
                proj_v(1)
                proj_qk(psF, wk_sb, slice(384, 512), ev_k)
        for i, (h, w, j) in enumerate(seq):
            ctx_step(i)
            if i + LEAD < NSTEP:
                sc_step(i + LEAD)
                exp_step(i + LEAD, eng=exp_eng(i + LEAD))
            if j == ST - 1 and i != NSTEP - 1:
                norm_evict(h, w)
            for f in sched.pop(i, []):
                f()
        assert not sched, f"unemitted filler slots: {sorted(sched)}"

        # ---- tail: last unit's norm pipelined with the h2 transpose and
        # the out-projection of qt 12..15. The ACT norm copies are emitted
        # qq-pairwise so each transpose piece starts as soon as its two
        # ctx_sb subtiles exist; qt12/13 only need transpose piece 0.
        pc = ctx_ps.pop((2, NW - 1))
        v3 = pc.rearrange("p (qq c) -> p qq c", c=DH + 1)
        rcp = work.tile([128, 4], F32, tag="rcp", name="rcp")
        nc.vector.reciprocal_approx_fast(
            out=rcp, in_=v3[:, :, DH : DH + 1].squeeze(-1))
        qt0 = (NW - 1) * 4
        # halves on both engines concurrently: DVE mul (qq0,1) + ACT scaled
        # copies (qq2,3) — the first transpose piece only needs qq0,1
        nc.vector.tensor_mul(
            ctx_sb[:, qt0 : qt0 + 2, 2 * DH : DH3],
            v3[:, 0:2, 0:DH],
            rcp[:, 0:2].unsqueeze(-1).broadcast_to([128, 2, DH]),
        )
        for qq in (2, 3):
            nc.scalar.activation(
                ctx_sb[:, qt0 + qq, 2 * DH : DH3],
                v3[:, qq, 0:DH],
                mybir.ActivationFunctionType.Copy,
                scale=rcp[:, qq : qq + 1],
            )
        trans_c(NW - 1, 0)
        outproj_qt(12, eng="dve")
        trans_c(NW - 1, 1)
        outproj_qt(13, eng="act")
        outproj_qt(14, eng="both", bfirst=True, dma="pool")
        outproj_qt(15, eng="rboth", bfirst=True)

    nc.compile()
    return nc


def _w_rearrange(w):
    """[768, M] -> [128, 6*M] bf16: row p holds w[kt*128+p, :] for kt=0..5,
    so the device DMA is one contiguous segment per partition."""
    import ml_dtypes

    d, m = w.shape
    kt = d // 128
    return np.ascontiguousarray(
        w.reshape(kt, 128, m).transpose(1, 0, 2).reshape(128, kt * m)
    ).astype(ml_dtypes.bfloat16)


def _bias_block(bq, bk, col):
    # [128, 4]: col0 = bk heads01, col1 = bk head2 (rows 0:64),
    # col2 = bq heads01 (pre-scaled), col3 = bq head2 at rows 64:128
    blk = np.zeros((128, 4), np.float32)
    blk[:, 0] = bk[col : col + 128]
    blk[0:64, 1] = bk[col + 128 : col + 192]
    blk[:, 2] = bq[col : col + 128] * np.float32(0.125)
    blk[64:128, 3] = bq[col + 128 : col + 192] * np.float32(0.125)
    return blk


def _prep_in_maps(inputs):
    import ml_dtypes

    bf16 = ml_dtypes.bfloat16
    x = np.asarray(inputs["x"], dtype=np.float32)
    Wq = np.asarray(inputs["Wq"], dtype=np.float32)
    Wk = np.asarray(inputs["Wk"], dtype=np.float32)
    Wv = np.asarray(inputs["Wv"], dtype=np.float32)
    Wo = np.asarray(inputs["Wo"], dtype=np.float32)
    bq = np.asarray(inputs["bq"], dtype=np.float32)
    bk = np.asarray(inputs["bk"], dtype=np.float32)
    ident = np.eye(128, dtype=np.float32).astype(bf16)

    in_maps = []
    for c in range(NCORES):
        b = c // 4
        col = (c % 4) * DH3
        sl = slice(col, col + DH3)
        in_maps.append(
            {
                "xT": np.ascontiguousarray(x[b].T).astype(bf16),
                "wq": _w_rearrange(Wq[:, col : col + 128] * np.float32(0.125)),
                "wk": _w_rearrange(Wk[:, col : col + 128]),
                "wkq2": _w_rearrange(np.concatenate(
                    [
                        Wk[:, col + 128 : col + 192],
                        Wq[:, col + 128 : col + 192] * np.float32(0.125),
                    ],
                    axis=1,
                )),
                "wv": _w_rearrange(Wv[:, sl]),
                "wo": np.ascontiguousarray(Wo[sl, :]).astype(bf16),
                "bias": _bias_block(bq, bk, col),
                "ident": ident,
            }
        )
    return in_maps


def _combine(results, inputs):
    Wo = np.asarray(inputs["Wo"], dtype=np.float32)
    bv = np.asarray(inputs["bv"], dtype=np.float32)
    bo = np.asarray(inputs["bo"], dtype=np.float32)
    base = bv @ Wo + bo  # [D]
    out = np.empty((B, S, D), dtype=np.float32)
    for b in range(B):
        acc = results[4 * b]["out"].astype(np.float32)
        for c in range(4 * b + 1, 4 * b + 4):
            acc = acc + results[c]["out"].astype(np.float32)
        out[b] = acc + base
    return out


def run(inputs, trace: bool = False):
    """Run the 8-core kernel; returns (output, BassKernelResults)."""
    global _CACHED_NC
    if _CACHED_NC is None:
        _CACHED_NC = _build_nc()
    in_maps = _prep_in_maps(inputs)
    try:
        res = run_bass_kernel_spmd(
            _CACHED_NC, in_maps, core_ids=list(range(NCORES)), trace=trace
        )
    except ModuleNotFoundError:
        import os

        os.environ["BASS_NEVER_TRACE"] = "1"
        res = run_bass_kernel_spmd(
            _CACHED_NC, in_maps, core_ids=list(range(NCORES)), trace=False
        )
    return _combine(res.results, inputs), res


def kernel(**inputs) -> np.ndarray:
    out, _ = run(inputs)
    return out


# revision 46
# speedup vs baseline: 1.1693x; 1.0012x over previous
"""MultiHeadAttention Trainium2 Bass kernel, v3.

Problem: B=2, S=2048, D=768, H=12 heads, head_dim=64.
    q = x@Wq+bq; k = x@Wk+bk; v = x@Wv+bv   (per-head split)
    out = softmax(q k^T / 8) v, heads merged, @ Wo + bo

Sharding (8 cores): core c handles batch b=c//4 and 3 heads (c%4)*3..+3
(Megatron attention: column-split of Wq/Wk/Wv, row-split of Wo). Each core
produces a partial [S, D] output; the host sums the 4 partials per batch and
adds (bv @ Wo + bo) once.

v3 over v2 (126.6us): v2 was paced by the ACT exp stream (96 x [128,1024]
exps at (1024+222)cyc @1.2GHz = 99.7us busy) plus a hard sc->exp psum
coupling (scores pool bufs=2 meant sc(i+2) waited exp(i)). v3:
 - offloads a tunable share of exp tiles to the DVE via a Schraudolph
   bit-trick exp (one tensor_scalar into an int16 tile whose bits ARE the
   bf16 exp; max rel err ~4.3%, pipeline rel err ~7e-3 vs the 2e-2 gate);
 - moves to 512-wide windows: 12 units x 16 key-tiles = 192 steps, scores
   psum [128,512] with bufs=4 so sc leads exp by 3 steps and exp leads ctx
   by 3 — the PE stream (~97us busy) becomes the pacer;
 - PSUM: 4x scores bank + 2x ctx bank + 2x filler bank = 8 banks; the
   out-projection borrows scores-ring slots (512+256 col chunks).

dtypes: moving-operand dtype sets matmul speed (f32r needs N>=256 for
1cyc/row; bf16 is 1cyc/row at any N). qT/kT stay f32r (full fp32 data);
x/v/ctx/Wv/Wo/identity are bf16 (small-N matmuls).

kernel(**inputs) takes FULL unsharded inputs and returns the FULL output.
"""

import numpy as np

import concourse.bass as bass
import concourse.mybir as mybir
import concourse.tile as tile
from concourse import bacc
from concourse.bass_utils import run_bass_kernel_spmd

F32 = mybir.dt.float32
F32R = mybir.dt.float32r
BF16 = mybir.dt.bfloat16
I16 = mybir.dt.int16

# Schraudolph exp on DVE: i16 = rint(s*SCH_C1 + SCH_C2); bitcast i16->bf16
# gives 2^n*(1+f) ~ exp(s) with max rel err ~4.3% (geometrically centered).
# DVE f32->i16 conversion is round-to-nearest (verified on hw).
SCH_C1 = 1.4426950408889634 * 128.0
SCH_C2 = 127.0 * 128.0 - 7.70

B, S, D = 2, 2048, 768
H, DH = 12, 64
NCORES = 8
HPC = 3                # heads per core
DH3 = HPC * DH         # 192 (per-core slice of the model dim)
KT = D // 128          # 6 contraction tiles for D
ST = S // 128          # 16 sequence tiles
GW = 512               # attention q-window width
NW = S // GW           # 4 windows
NSTEP = HPC * NW * ST  # 192 pipeline steps

_CACHED_NC = None


def _build_nc(debug: bool = False) -> bass.Bass:
    nc = bacc.Bacc()

    xT = nc.dram_tensor("xT", [D, S], BF16, kind="ExternalInput")
    wq = nc.dram_tensor("wq", [128, KT * 128], BF16, kind="ExternalInput")
    wk = nc.dram_tensor("wk", [128, KT * 128], BF16, kind="ExternalInput")
    wkq2 = nc.dram_tensor("wkq2", [128, KT * 128], BF16, kind="ExternalInput")
    wv = nc.dram_tensor("wv", [128, KT * DH3], BF16, kind="ExternalInput")
    wo = nc.dram_tensor("wo", [DH3, D], BF16, kind="ExternalInput")
    bias = nc.dram_tensor("bias", [128, 4], F32, kind="ExternalInput")
    ident = nc.dram_tensor("ident", [128, 128], BF16, kind="ExternalInput")
    out = nc.dram_tensor("out", [S, D], BF16, kind="ExternalOutput")

    with (
        tile.TileContext(nc) as tc,
        tc.tile_pool(name="big", bufs=1) as big,
        tc.tile_pool(name="work", bufs=2) as work,
        tc.tile_pool(name="expp", bufs=4) as expp,
        tc.tile_pool(name="outp", bufs=4) as outp,
        tc.tile_pool(name="psS", bufs=4, space="PSUM") as psS,
        tc.tile_pool(name="psB", bufs=2, space="PSUM") as psB,
        tc.tile_pool(name="psF", bufs=2, space="PSUM") as psF,
    ):
        # ---- persistent SBUF tensors ----
        x_sb = big.tile([128, KT, S], BF16)          # xT: [p, ktile, s]
        wq_sb = big.tile([128, KT, 128], BF16)
        wk_sb = big.tile([128, KT, 128], BF16)
        wkq2_sb = big.tile([128, KT, 128], BF16)     # [k_h2 | q_h2]
        wv_sb = big.tile([128, KT, DH3], BF16)
        woA_sb = big.tile([128, D], BF16)            # Wo rows 0..127
        woB_sb = big.tile([64, D], BF16)             # Wo rows 128..191
        bias_sb = big.tile([128, 4], F32)
        ident_sb = big.tile([128, 128], BF16)
        qTA = big.tile([128, S], F32R)               # q^T heads 0,1
        kTA = big.tile([128, S], F32R)
        qTB = big.tile([128, S], F32R)               # head 2 in rows 64:128
        kTB = big.tile([128, S], F32R)
        v_sb = big.tile([128, ST, HPC, DH + 1], BF16)  # v rows + ones col
        ctx_sb = big.tile([128, ST, DH3], BF16)      # [q-part, qt, h*64+d]
        ctxTA = big.tile([128, S], BF16)             # ctx^T heads 0,1
        ctxTB = big.tile([64, S], BF16)              # ctx^T head 2

        # ---- DMA loads ----
        # single sync queue: the serial (~360GB/s) DMA bus moves bytes in
        # exactly the order the pipeline consumes them. First window needs
        # wq + x(0:512) + wk + wv (v fillers start at step 0).
        nc.sync.dma_start(out=wq_sb, in_=wq.rearrange("p (kt m) -> p kt m", kt=KT))
        for c in range(2):
            cs = slice(c * 256, (c + 1) * 256)
            nc.sync.dma_start(
                out=x_sb[:, :, cs],
                in_=xT[:, cs].rearrange("(kt p) q -> p kt q", p=128),
            )
        nc.scalar.dma_start(out=bias_sb, in_=bias[:, :])
        nc.sync.dma_start(out=wk_sb, in_=wk.rearrange("p (kt m) -> p kt m", kt=KT))
        nc.sync.dma_start(out=wv_sb, in_=wv.rearrange("p (kt m) -> p kt m", kt=KT))
        nc.sync.dma_start(
            out=x_sb[:, :, 512:1024],
            in_=xT[:, 512:1024].rearrange("(kt p) q -> p kt q", p=128),
        )
        nc.sync.dma_start(out=wkq2_sb, in_=wkq2.rearrange("p (kt m) -> p kt m", kt=KT))
        for c in range(2, 4):
            cs = slice(c * 512, (c + 1) * 512)
            nc.sync.dma_start(
                out=x_sb[:, :, cs],
                in_=xT[:, cs].rearrange("(kt p) q -> p kt q", p=128),
            )
        nc.sync.dma_start(out=woA_sb, in_=wo[0:128, :])
        nc.sync.dma_start(out=woB_sb, in_=wo[128:DH3, :])
        nc.sync.dma_start(out=ident_sb, in_=ident[:, :])
        nc.vector.memset(v_sb[:, :, :, DH : DH + 1], 1.0)

        # preload the Exp activation table during the DMA wait (the implicit
        # LoadActFuncSet lands before this dummy, off the critical path)
        dum_in = big.tile([1, 1], F32)
        dum_out = big.tile([1, 1], F32)
        nc.vector.memset(dum_in, 0.0)
        nc.scalar.activation(dum_out, dum_in, mybir.ActivationFunctionType.Exp)

        # warm the PE p-state during the x DMA wait: the Tensor engine ramps
        # 0.65 -> 1.2 -> 2.4 GHz only after ~3us of continuous execution and
        # the ramp resets when PE idles; burn the DMA wait on throwaway
        # matmuls sized to end as the first x chunk lands (~1.6us).
        warm = big.tile([128, 512], BF16)
        nc.gpsimd.memset(warm, 0.0)   # Pool is idle; DVE memsets would delay
        for _ in range(9):
            pw = psS.tile([128, 512], F32, tag="s", name="ps_warm")
            nc.tensor.matmul(pw, lhsT=warm[:, 0:128], rhs=warm, start=True,
                             stop=True)

        # ---- projection helpers ----
        def proj_qk(pool, w_sb, cs, evict):
            n = cs.stop - cs.start
            ps = pool.tile([128, n], F32, tag=("s" if pool is psS else "f"),
                           name="ps_qk", padded_shape=None)
            for kt in range(KT):
                nc.tensor.matmul(
                    ps,
                    lhsT=w_sb[:, kt, :],
                    rhs=x_sb[:, kt, cs],
                    start=(kt == 0),
                    stop=(kt == KT - 1),
                )
            evict(ps, cs)

        def ev_k(ps, cs):
            nc.vector.tensor_scalar_add(kTA[:, cs], ps, bias_sb[:, 0:1])

        def ev_q(ps, cs):
            nc.vector.tensor_scalar_add(qTA[:, cs], ps, bias_sb[:, 2:3])

        def ev_kq2(ps, cs):
            # psum rows 0:64 = k_h2 (up-shift to 64:128), rows 64:128 = q_h2
            nc.vector.tensor_scalar_add(kTB[64:128, cs], ps[0:64, :], bias_sb[0:64, 1:2])
            nc.vector.tensor_scalar_add(qTB[64:128, cs], ps[64:128, :], bias_sb[64:128, 3:4])

        def proj_v(st):
            # all 3 heads' v for one seq tile: [128, 192] psum, one strided
            # eviction into v_sb (skipping the ones column)
            ss = slice(st * 128, (st + 1) * 128)
            ps_v = psF.tile([128, DH3], F32, tag="f", name="ps_v")
            for kt in range(KT):
                nc.tensor.matmul(
                    ps_v,
                    lhsT=x_sb[:, kt, ss],
                    rhs=wv_sb[:, kt, :],
                    start=(kt == 0),
                    stop=(kt == KT - 1),
                )
            nc.vector.tensor_copy(
                v_sb[:, st, :, 0:DH],
                ps_v.rearrange("p (h d) -> p h d", h=HPC),
            )

        # head h (q/k)^T slices: heads 0,1 in kTA/qTA rows 0:64 / 64:128,
        # head 2 in kTB/qTB rows 64:128.
        def kq_rows(h):
            if h == 0:
                return kTA, qTA, slice(0, 64)
            if h == 1:
                return kTA, qTA, slice(64, 128)
            return kTB, qTB, slice(64, 128)

        # ---- attention pipeline pieces ----
        # units of (head, 512-q-window); per unit j walks the 16 key tiles.
        # Heads in order (0,1,2) per window so h0h1's combined transpose of
        # the LAST window fires before the final unit (h2) completes.
        units = [(h, w) for w in range(NW) for h in range(HPC)]
        seq = [(h, w, j) for (h, w) in units for j in range(ST)]

        sc_tiles = {}   # step -> scores psum tile
        et_tiles = {}   # step -> exp tile
        ctx_ps = {}     # (h, w) -> ctx psum tile

        def sc_step(i):
            h, w, j = seq[i]
            kk, qq, rows = kq_rows(h)
            ps = psS.tile([128, GW], F32, tag="s", name="ps_sc")
            nc.tensor.matmul(
                ps,
                lhsT=kk[rows, j * 128 : (j + 1) * 128],
                rhs=qq[rows, w * GW : (w + 1) * GW],
                start=True,
                stop=True,
            )
            sc_tiles[i] = ps

        def exp_step(i, eng="act"):
            ps = sc_tiles.pop(i)
            et = expp.tile([128, GW], BF16, tag="e", name="expT")
            if eng == "dve":
                # approximate exp on DVE (bit-trick), freeing ACT: the int16
                # write IS the bf16 exp tile by bitcast.
                nc.vector.tensor_scalar(
                    out=et.bitcast(I16), in0=ps,
                    scalar1=SCH_C1, scalar2=SCH_C2,
                    op0=mybir.AluOpType.mult, op1=mybir.AluOpType.add,
                )
            else:
                nc.scalar.activation(et, ps, mybir.ActivationFunctionType.Exp)
            et_tiles[i] = et

        def ctx_step(i):
            h, w, j = seq[i]
            et = et_tiles.pop(i)
            e0 = 0
            key = (h, w)
            if key not in ctx_ps:
                ctx_ps[key] = psB.tile([128, 4 * (DH + 1)], F32, tag="b",
                                       name="ps_ctx")
            pc = ctx_ps[key]
            for qq in range(4):
                # start marks the WHOLE 2KB psum bank pending-zero, so only
                # the first write of the bank's group may set it.
                nc.tensor.matmul(
                    pc[:, qq * (DH + 1) : (qq + 1) * (DH + 1)],
                    lhsT=et[:, (e0 + qq) * 128 : (e0 + qq + 1) * 128],
                    rhs=v_sb[:, j, h, :],
                    start=(j == 0 and qq == 0),
                    stop=(j == ST - 1 and qq == 3),
                    skip_group_check=True,
                )

        def norm_evict(h, w, last=False):
            # psum [128, 4*(65)]: per qq, cols 0:64 = ctx, col 64 = denom.
            pc = ctx_ps.pop((h, w))
            v3 = pc.rearrange("p (qq c) -> p qq c", c=DH + 1)
            rcp = work.tile([128, 4], F32, tag="rcp", name="rcp")
            nc.vector.reciprocal_approx_fast(
                out=rcp, in_=v3[:, :, DH : DH + 1].squeeze(-1))
            qt0 = w * 4
            if last:
                # final unit: ACT is idle after the last exp — normalize
                # there (Copy with per-partition scale), one qq per instr,
                # in parallel with DVE work
                for qq in range(4):
                    nc.scalar.activation(
                        ctx_sb[:, qt0 + qq, h * DH : (h + 1) * DH],
                        v3[:, qq, 0:DH],
                        mybir.ActivationFunctionType.Copy,
                        scale=rcp[:, qq : qq + 1],
                    )
                return
            nc.vector.tensor_mul(
                ctx_sb[:, qt0 : qt0 + 4, h * DH : (h + 1) * DH],
                v3[:, :, 0:DH],
                rcp.unsqueeze(-1).broadcast_to([128, 4, DH]),
            )

        # ---- transpose + out-projection ----
        def trans_ab(w, piece):
            # heads 0+1 combined: ctx_sb[:, qt, 0:128] is [q, h0 d | h1 d];
            # one [128,128] PE transpose per qt covers both heads. Two qt
            # per 256-col psF piece.
            pt = psF.tile([128, 256], BF16, tag="f", name="ps_t")
            for qq in (0, 1):
                qt = w * 4 + piece * 2 + qq
                nc.tensor.transpose(
                    pt[:, qq * 128 : (qq + 1) * 128],
                    ctx_sb[:, qt, 0:128], ident_sb,
                )
            cs = slice(w * GW + piece * 256, w * GW + (piece + 1) * 256)
            nc.vector.tensor_copy(ctxTA[:, cs], pt)

        def trans_c(w, piece):
            # head 2 -> ctxTB rows 0:64
            pt = psF.tile([64, 256], BF16, tag="f", name="ps_t2")
            for qq in (0, 1):
                qt = w * 4 + piece * 2 + qq
                nc.tensor.transpose(
                    pt[0:64, qq * 128 : (qq + 1) * 128],
                    ctx_sb[:, qt, 2 * DH : DH3], ident_sb,
                )
            cs = slice(w * GW + piece * 256, w * GW + (piece + 1) * 256)
            nc.vector.tensor_copy(ctxTB[0:64, cs], pt[0:64, :])

        o_tiles = {}

        def outproj_chunk(qt, ci, eng="dve", bfirst=False, dma=None):
            # one psS-ring chunk (512 or 256 cols); two matmuls (K=192 over
            # ctxTA 128 rows + ctxTB 64 rows); evict into the qt's [128,768]
            # staging tile; the SECOND chunk issues a single DMA for the qt
            # (each DMA costs ~625ns serialized HWDGE overhead, so one per
            # qt). Chunk-granular so the psS ring pressure per step stays
            # at one slot.
            ss = slice(qt * 128, (qt + 1) * 128)
            osl = (slice(0, 512), slice(512, D))[ci]
            if qt not in o_tiles:
                o_tiles[qt] = outp.tile([128, D], BF16, tag="o", name="o_sb")
            o_sb = o_tiles[qt]
            n = osl.stop - osl.start
            po = psS.tile([128, n], F32, tag="s", name="ps_o")
            pairs = [(ctxTA, woA_sb), (ctxTB, woB_sb)]
            if bfirst:
                pairs.reverse()
            for pi, (ct, wo_sb) in enumerate(pairs):
                nc.tensor.matmul(
                    po, lhsT=ct[:, qt * 128 : (qt + 1) * 128],
                    rhs=wo_sb[:, osl], start=(pi == 0), stop=(pi == 1),
                )
            if eng == "act":
                nc.scalar.activation(o_sb[:, osl], po,
                                     mybir.ActivationFunctionType.Copy)
            else:
                nc.vector.tensor_copy(o_sb[:, osl], po)
            if dma == "split":
                # final qt: DMA each chunk as its eviction lands so the
                # last serialized HWDGE+transfer is the small 256-col one
                nc.sync.dma_start(out=out[ss, osl], in_=o_sb[:, osl])
                if ci == 1:
                    del o_tiles[qt]
            elif ci == 1:
                del o_tiles[qt]
                if dma == "pool":
                    # SWDGE path — bypasses the single-slot HWDGE so the
                    # final DMAs drain in parallel
                    nc.gpsimd.dma_start(out=out[ss, :], in_=o_sb)
                else:
                    nc.sync.dma_start(out=out[ss, :], in_=o_sb)

        def outproj_qt(qt, eng="dve", bfirst=False, dma=None):
            if eng == "both":
                e0, e1 = "dve", "act"
            elif eng == "rboth":
                e0, e1 = "act", "dve"
            else:
                e0, e1 = eng, eng
            outproj_chunk(qt, 0, e0, bfirst, dma)
            outproj_chunk(qt, 1, e1, bfirst, dma)

        # ---- startup: projections needed before the exp stream starts ----
        # q w0 in two 256-col pieces pipelined behind the x DMA pieces;
        # k j0 next; the rest stream in as fillers.
        proj_qk(psS, wq_sb, slice(0, 256), ev_q)
        proj_qk(psS, wq_sb, slice(256, 512), ev_q)
        proj_qk(psS, wk_sb, slice(0, 128), ev_k)

        # ---- per-step filler schedule ----
        # sched[i] = closures emitted at pipeline step i. EMISSION-ORDER
        # LAW: a consumer emitted before its producer gets NO dependency
        # edge and reads garbage. Need-by rules:
        #   k chunk j        -> before step j-2 (sc(j) emitted at step j-3)
        #   v tile st        -> before step st (ctx(h0-unit, j=st))
        #   kq2 chunk c      -> k2 col j read at step 32+j; q2 w read at
        #                       step 48w+32 (chunks 4w..4w+3)
        #   q w chunks       -> before step 48w-3 (sc emission lead)
        #   trans w          -> after norm(1,w) at step 48w+31 (ab) and
        #                       norm(2,w) at step 48w+47 (c)
        #   outproj qt of w  -> after trans w
        def mk_qk(w_sb, cs, evict):
            return lambda: proj_qk(psF, w_sb, cs, evict)

        k2cs = [slice(128 * i, 128 * (i + 1)) for i in range(16)]
        sched = {}

        def put(slot, f):
            sched.setdefault(slot, []).append(f)

        for j in range(4, 16, 2):  # k chunks for sc j=4..15, 256 wide
            put(j - 4, mk_qk(wk_sb, slice(j * 128, (j + 2) * 128), ev_k))
        for st in range(2, 16):  # v tiles (all 3 heads each)
            put(min(st - 2, 13), lambda st=st: proj_v(st))
        # kq2: q2 w0 (chunks 0..3) read at step 29; k2 col j at step 29+j —
        # spread the later chunks to fill the otherwise-empty steps 36..42
        K2SLOTS = [15, 17, 20, 24, 28, 32, 36, 40]
        for c in range(8):
            put(K2SLOTS[c], mk_qk(wkq2_sb, slice(256 * c, 256 * (c + 1)),
                                  ev_kq2))
        for w in range(1, NW):   # q for windows 1..3: 2 chunks of 256
            for c in range(2):
                cs = slice(w * GW + c * 256, w * GW + (c + 1) * 256)
                put(48 * w - 16 + 5 * c, mk_qk(wq_sb, cs, ev_q))
        # transposes + out-projection per window (windows 0..2 here; the
        # last window's h2 part and out-proj form the tail). Out-proj
        # chunks are spread so the filler stream reaches into the
        # otherwise-empty final unit.
        for w in range(NW):
            base = 48 * w
            for piece in range(2):
                # fill the otherwise-empty window-end steps 44..47 (norm of
                # (1,w) lands at step 48w+31, so 44/46 are safe); the last
                # window's blocks go right after its h1 norm instead.
                put(base + (44 + 2 * piece if w < NW - 1 else 32 + 2 * piece),
                    lambda w=w, p=piece: trans_ab(w, p))
                if w < NW - 1:
                    put(base + 48 + piece, lambda w=w, p=piece: trans_c(w, p))
            if w < NW - 1:
                step = 6 if w == NW - 2 else 4
                for kc in range(8):
                    qt, ci = w * 4 + kc // 2, kc % 2
                    put(base + 50 + step * kc,
                        lambda qt=qt, ci=ci, kc=kc:
                            outproj_chunk(qt, ci,
                                          eng=("act" if (kc // 2) % 2 else "dve")))

        # exp-engine assignment: DVE takes a share sized to each
        # region's DVE eviction load.
        def exp_eng(i):
            if i < 16:
                return "act"        # unit 0: DVE busy with v/k evicts
            if i < 48:
                return "dve" if i % 4 == 1 else "act"
            if i >= 189:
                return "dve" if i == 190 else "act"  # final steps gate the tail
            return "dve" if (i % 16) in (1, 3, 5, 8, 10, 12, 14) else "act"

        # ---- main pipelined emission ----
        # sc/exp lead ctx by 3 steps (psS ring depth 4); exp is emitted
        # right after its sc so it sits early in its engine queue.
        LEAD = 3
        for i in range(LEAD):
            sc_step(i)
            exp_step(i, eng=exp_eng(i))
            if i == 0:
                proj_qk(psF, wk_sb, slice(128, 256), ev_k)
            elif i == 1:
                proj_qk(psF, wk_sb, slice(256, 384), ev_k)
                proj_v(0)
            else:
                proj_v(1)
                proj_qk(psF, wk_sb, slice(384, 512), ev_k)
        for i, (h, w, j) in enumerate(seq):
            ctx_step(i)
            if i + LEAD < NSTEP:
                sc_step(i + LEAD)
                exp_step(i + LEAD, eng=exp_eng(i + LEAD))
            if j == ST - 1 and i != NSTEP - 1:
                norm_evict(h, w)
            for f in sched.pop(i, []):
                f()
        assert not sched, f"unemitted filler slots: {sorted(sched)}"

        # ---- tail: last unit's norm pipelined with the h2 transpose and
        # the out-projection of qt 12..15. The ACT norm copies are emitted
        # qq-pairwise so each transpose piece starts as soon as its two
        # ctx_sb subtiles exist; qt12/13 only need transpose piece 0.
        pc = ctx_ps.pop((2, NW - 1))
        v3 = pc.rearrange("p (qq c) -> p qq c", c=DH + 1)
        rcp = work.tile([128, 4], F32, tag="rcp", name="rcp")
        nc.vector.reciprocal_approx_fast(
            out=rcp, in_=v3[:, :, DH : DH + 1].squeeze(-1))
        qt0 = (NW - 1) * 4
        # halves on both engines concurrently: DVE mul (qq0,1) + ACT scaled
        # copies (qq2,3) — the first transpose piece only needs qq0,1
        nc.vector.tensor_mul(
            ctx_sb[:, qt0 : qt0 + 2, 2 * DH : DH3],
            v3[:, 0:2, 0:DH],
            rcp[:, 0:2].unsqueeze(-1).broadcast_to([128, 2, DH]),
        )
        for qq in (2, 3):
            nc.scalar.activation(
                ctx_sb[:, qt0 + qq, 2 * DH : DH3],
                v3[:, qq, 0:DH],
                mybir.ActivationFunctionType.Copy,
                scale=rcp[:, qq : qq + 1],
            )
        trans_c(NW - 1, 0)
        outproj_qt(12, eng="dve")
        trans_c(NW - 1, 1)
        outproj_qt(13, eng="act")
        outproj_qt(14, eng="both", bfirst=True, dma="pool")
        outproj_qt(15, eng="rboth", bfirst=True)

    nc.compile()
    return nc


def _w_rearrange(w):
    """[768, M] -> [128, 6*M] bf16: row p holds w[kt*128+p, :] for kt=0..5,
    so the device DMA is one contiguous segment per partition."""
    import ml_dtypes

    d, m = w.shape
    kt = d // 128
    return np.ascontiguousarray(
        w.reshape(kt, 128, m).transpose(1, 0, 2).reshape(128, kt * m)
    ).astype(ml_dtypes.bfloat16)


def _bias_block(bq, bk, col):
    # [128, 4]: col0 = bk heads01, col1 = bk head2 (rows 0:64),
    # col2 = bq heads01 (pre-scaled), col3 = bq head2 at rows 64:128
    blk = np.zeros((128, 4), np.float32)
    blk[:, 0] = bk[col : col + 128]
    blk[0:64, 1] = bk[col + 128 : col + 192]
    blk[:, 2] = bq[col : col + 128] * np.float32(0.125)
    blk[64:128, 3] = bq[col + 128 : col + 192] * np.float32(0.125)
    return blk


def _prep_in_maps(inputs):
    import ml_dtypes

    bf16 = ml_dtypes.bfloat16
    x = np.asarray(inputs["x"], dtype=np.float32)
    Wq = np.asarray(inputs["Wq"], dtype=np.float32)
    Wk = np.asarray(inputs["Wk"], dtype=np.float32)
    Wv = np.asarray(inputs["Wv"], dtype=np.float32)
    Wo = np.asarray(inputs["Wo"], dtype=np.float32)
    bq = np.asarray(inputs["bq"], dtype=np.float32)
    bk = np.asarray(inputs["bk"], dtype=np.float32)
    ident = np.eye(128, dtype=np.float32).astype(bf16)

    in_maps = []
    for c in range(NCORES):
        b = c // 4
        col = (c % 4) * DH3
        sl = slice(col, col + DH3)
        in_maps.append(
            {
                "xT": np.ascontiguousarray(x[b].T).astype(bf16),
                "wq": _w_rearrange(Wq[:, col : col + 128] * np.float32(0.125)),
                "wk": _w_rearrange(Wk[:, col : col + 128]),
                "wkq2": _w_rearrange(np.concatenate(
                    [
                        Wk[:, col + 128 : col + 192],
                        Wq[:, col + 128 : col + 192] * np.float32(0.125),
                    ],
                    axis=1,
                )),
                "wv": _w_rearrange(Wv[:, sl]),
                "wo": np.ascontiguousarray(Wo[sl, :]).astype(bf16),
                "bias": _bias_block(bq, bk, col),
                "ident": ident,
            }
        )
    return in_maps


def _combine(results, inputs):
    Wo = np.asarray(inputs["Wo"], dtype=np.float32)
    bv = np.asarray(inputs["bv"], dtype=np.float32)
    bo = np.asarray(inputs["bo"], dtype=np.float32)
    base = bv @ Wo + bo  # [D]
    out = np.empty((B, S, D), dtype=np.float32)
    for b in range(B):
        acc = results[4 * b]["out"].astype(np.float32)
        for c in range(4 * b + 1, 4 * b + 4):
            acc = acc + results[c]["out"].astype(np.float32)
        out[b] = acc + base
    return out


def run(inputs, trace: bool = False):
    """Run the 8-core kernel; returns (output, BassKernelResults)."""
    global _CACHED_NC
    if _CACHED_NC is None:
        _CACHED_NC = _build_nc()
    in_maps = _prep_in_maps(inputs)
    try:
        res = run_bass_kernel_spmd(
            _CACHED_NC, in_maps, core_ids=list(range(NCORES)), trace=trace
        )
    except ModuleNotFoundError:
        import os

        os.environ["BASS_NEVER_TRACE"] = "1"
        res = run_bass_kernel_spmd(
            _CACHED_NC, in_maps, core_ids=list(range(NCORES)), trace=False
        )
    return _combine(res.results, inputs), res


def kernel(**inputs) -> np.ndarray:
    out, _ = run(inputs)
    return out
